# revision 20
# baseline (speedup 1.0000x reference)
"""ChannelWiseFloat8GroupedLinear — expert-parallel Trainium2 Bass kernel.

Problem: x [8192, 1024] f32, weight [8*1024, 1024] f32, tokens_per_expert
[8] int32 (uniform 1024).  out[t, d] = x_dq @ w_dq[e(t)].T in bf16, where
x is fp8-e4m3fn quant-dequantized per token row and w per expert block.

Sharding: expert-parallel over 8 NeuronCores.  Tokens are contiguous per
expert (cumsum offsets), so core e owns x rows [1024e, 1024e+1024) and
expert e's weight block — no cross-core communication.

Device math: the reference quantizes to OCP e4m3fn (max 448); TRN2's
fp8_e4m3 tops out at 240.  Quantizing with r = 224/amax instead of
448/amax lands on the halved e4m3fn grid, which TRN e4m3 represents
exactly (up to a negligible subnormal-spacing difference), and the x4
is folded into the output scale m[t] = amax_x[t]*amax_w/(448*448/4).
The fp8 matmul accumulates exact products in f32 PSUM, so the result
matches the reference to ~f32 rounding before the final bf16 cast.
"""

import numpy as np
import ml_dtypes

P = 128
TPE = 1024   # tokens per expert (= T // ne, uniform)
DIN = 1024
DOUT = 1024
NE = 8
NT = TPE // P    # 8 token tiles per core
ND = DOUT // P   # 8 dout tiles per core
NK = DIN // P    # 8 contraction tiles
E4M3_MAX = 448.0
EPS = 1e-12

_CACHE = {}


def _axon_device_reset():
    """Best-effort reset of the axon-tunneled NeuronCores after an
    NRT_EXEC_UNIT_UNRECOVERABLE wedge (observed rarely; a reset recovers)."""
    try:
        import ctypes

        import jax

        jax.devices()
        lib = ctypes.CDLL("/opt/axon/libaxon_pjrt.so")
        if hasattr(lib, "axon_reset"):
            lib.axon_reset.restype = ctypes.c_int64
            lib.axon_reset()
    except Exception:
        pass


def _build_nc():
    """Build + compile the single-core Bass program (run SPMD on 8 cores)."""
    import concourse.mybir as mybir
    import concourse.tile as tile
    from concourse import bacc, bass_isa
    from concourse.masks import make_identity

    dt = mybir.dt
    X = mybir.AxisListType.X
    ALU = mybir.AluOpType

    nc = bacc.Bacc("TRN2", target_bir_lowering=False, debug=False)
    x_t = nc.dram_tensor("x", [TPE, DIN], dt.float32, kind="ExternalInput")
    w_t = nc.dram_tensor("w", [DOUT, DIN], dt.float32, kind="ExternalInput")
    o_t = nc.dram_tensor("o", [TPE, DOUT], dt.bfloat16, kind="ExternalOutput")

    x_d = x_t.ap().rearrange("(tt p) k -> p tt k", p=P)   # [128, 8, 1024]
    w_d = w_t.ap().rearrange("(dd p) k -> p dd k", p=P)
    o_d = o_t.ap().rearrange("(tt p) d -> p tt d", p=P)

    with tile.TileContext(nc) as tc:
        with (
            tc.tile_pool(name="const", bufs=1) as const,
            tc.tile_pool(name="big", bufs=1) as big,
            tc.tile_pool(name="small", bufs=1) as small,
            tc.tile_pool(name="outp", bufs=3) as outp,
            tc.tile_pool(name="pt", bufs=2, space="PSUM") as pt,
            tc.tile_pool(name="pm", bufs=2, space="PSUM") as pm,
        ):
            # persistent buffers
            x_sb = big.tile([P, NT, DIN], dt.float32, tag="x_sb")
            w_sb = big.tile([P, ND, DIN], dt.float32, tag="w_sb")
            qx = big.tile([P, NT, DIN], dt.float8e4, tag="qx")
            wT = big.tile([P, NK, ND, P], dt.float32, tag="wT")
            qxT = big.tile([P, NK, NT, P], dt.float8e4, tag="qxT")
            qwT = big.tile([P, NK, ND, P], dt.float8e4, tag="qwT")

            amw_parts = small.tile([P, ND], dt.float32, tag="amw_parts")
            amw_c = small.tile([P, 1], dt.float32, tag="amw_c")
            amw_g = small.tile([P, 1], dt.float32, tag="amw_g")
            inv_w = small.tile([P, 1], dt.float32, tag="inv_w")
            rw = small.tile([P, 1], dt.float32, tag="rw")
            cw = small.tile([P, 1], dt.float32, tag="cw")
            amx_parts = small.tile([P, NT], dt.float32, tag="amx_parts")
            amx_cl = small.tile([P, NT], dt.float32, tag="amx_cl")
            inv_x = small.tile([P, NT], dt.float32, tag="inv_x")
            rx = small.tile([P, NT], dt.float32, tag="rx")
            m_all = small.tile([P, NT], dt.float32, tag="m_all")

            # --- loads first: w first (its global amax gates the w pipeline).
            # 0.5MB chunks: smaller completion-receipt lag on each tile. ---
            for i in range(ND):
                nc.sync.dma_start(w_sb[:, i, :], w_d[:, i, :])
            for i in range(NT):
                nc.sync.dma_start(x_sb[:, i, :], x_d[:, i, :])

            # transpose identity (fp8: 1.0 is exactly representable)
            id_f32 = const.tile([P, P], dt.float32, tag="id32f")
            make_identity(nc, id_f32[:])
            id_fp8 = const.tile([P, P], dt.float8e4, tag="id8")
            nc.vector.tensor_copy(id_fp8[:], id_f32[:])

            # --- amax reduces first: they get engine priority so the
            # per-tile reductions run the moment each DMA lands ---
            for dd in range(ND):
                nc.vector.reduce_max(
                    amw_parts[:, dd : dd + 1],
                    w_sb[:, dd, :],
                    axis=X,
                    apply_absolute_value=True,
                )
            nc.vector.reduce_max(amw_c[:], amw_parts[:], axis=X)
            nc.vector.tensor_scalar_max(amw_c[:], amw_c[:], EPS)
            nc.gpsimd.partition_all_reduce(
                amw_g[:], amw_c[:], channels=P, reduce_op=bass_isa.ReduceOp.max
            )
            nc.vector.reciprocal(inv_w[:], amw_g[:])
            nc.vector.tensor_scalar_mul(rw[:], inv_w[:], E4M3_MAX / 2.0)
            nc.vector.tensor_scalar_mul(cw[:], amw_g[:], 4.0 / (E4M3_MAX * E4M3_MAX))


            # --- w: exact f32 transpose (PE transpose-mode) during the load
            # window (no amax dependency), staged to wT in f32 ---
            for dd in range(ND):
                pwf = pt.tile([P, NK, P], dt.float32, tag="pt")
                for kk in range(NK):
                    nc.tensor.transpose(
                        pwf[:, kk, :], w_sb[:, dd, kk * P : (kk + 1) * P], id_f32[:]
                    )
                nc.scalar.copy(wT[:, :, dd, :], pwf[:])

            # --- w quantize per kk, split DVE/ACT so the serial quant
            # chain halves and the main matmul unblocks sooner (DVE runs
            # fp32 tensor_scalar in 2-port mode ~0.75us vs ACT ~1.15us) ---
            for kk in range(NK):
                if kk % 2 == 0:
                    nc.vector.tensor_scalar_mul(
                        qwT[:, kk, :, :], wT[:, kk, :, :], rw[:]
                    )
                else:
                    nc.scalar.mul(qwT[:, kk, :, :], wT[:, kk, :, :], rw[:])

            # --- x: per-pair chain (amax -> scales -> quant -> PE transpose
            # -> evict), emission-ordered so engine priorities follow the
            # dependency chain ---
            for pair in range(4):
                t0 = 2 * pair
                sl = slice(t0, t0 + 2)
                for tt in (t0, t0 + 1):
                    nc.vector.reduce_max(
                        amx_parts[:, tt : tt + 1],
                        x_sb[:, tt, :],
                        axis=X,
                        apply_absolute_value=True,
                    )
                nc.vector.tensor_scalar_max(amx_cl[:, sl], amx_parts[:, sl], EPS)
                nc.vector.reciprocal(inv_x[:, sl], amx_cl[:, sl])
                nc.vector.tensor_scalar_mul(rx[:, sl], inv_x[:, sl], E4M3_MAX / 2.0)
                nc.vector.tensor_scalar(
                    m_all[:, sl], amx_cl[:, sl], cw[:], None, op0=ALU.mult
                )
                for tt in (t0, t0 + 1):
                    nc.vector.tensor_scalar_mul(
                        qx[:, tt, :], x_sb[:, tt, :], rx[:, tt : tt + 1]
                    )
                    pxf = pt.tile([P, NK, P], dt.float32, tag="pt")
                    for kk in range(NK):
                        nc.tensor.matmul(
                            pxf[:, kk, :],
                            lhsT=qx[:, tt, kk * P : (kk + 1) * P],
                            rhs=id_fp8[:],
                            start=True, stop=True,
                        )
                    if tt % 2 == 0:
                        nc.vector.tensor_copy(qxT[:, :, tt, :], pxf[:])
                    else:
                        nc.scalar.copy(qxT[:, :, tt, :], pxf[:])

            # --- main fp8 matmul: out[t,d] accumulated over k, DoubleRow
            # (2 fp8 weights per PE cell -> two k-tiles per pass, ~1.4x) ---
            DR = mybir.MatmulPerfMode.DoubleRow
            for tt in range(NT):
                po = pm.tile([P, DOUT], dt.float32, tag="pm")
                for kp in range(NK // 2):
                    ks = slice(2 * kp, 2 * kp + 2)
                    st, sp = kp == 0, kp == NK // 2 - 1
                    nc.tensor.matmul(
                        po[:, 0 : DOUT // 2],
                        lhsT=qxT[:, ks, tt, :],
                        rhs=qwT[:, ks, 0 : ND // 2, :],
                        start=st, stop=sp, perf_mode=DR,
                    )
                    nc.tensor.matmul(
                        po[:, DOUT // 2 : DOUT],
                        lhsT=qxT[:, ks, tt, :],
                        rhs=qwT[:, ks, ND // 2 : ND, :],
                        start=st, stop=sp, perf_mode=DR,
                    )
                ob = outp.tile([P, DOUT], dt.bfloat16, tag="ob")
                if tt % 2 == 0:
                    nc.vector.tensor_scalar_mul(ob[:], po[:], m_all[:, tt : tt + 1])
                else:
                    nc.scalar.mul(ob[:], po[:], m_all[:, tt : tt + 1])
                nc.sync.dma_start(o_d[:, tt, :], ob[:])

    nc.compile()
    return nc


def get_nc():
    if "nc" not in _CACHE:
        _CACHE["nc"] = _build_nc()
    return _CACHE["nc"]


def make_in_maps(x, weight):
    x = np.ascontiguousarray(np.asarray(x, dtype=np.float32))
    w = np.ascontiguousarray(np.asarray(weight, dtype=np.float32))
    return [
        {"x": x[TPE * e : TPE * (e + 1)], "w": w[DOUT * e : DOUT * (e + 1)]}
        for e in range(NE)
    ]


def _host_reference(x, weight, tokens_per_expert):
    """Exact numpy port of the reference — fallback for non-uniform routing."""
    x = np.asarray(x, dtype=np.float32)
    w = np.asarray(weight, dtype=np.float32)
    tpe = np.asarray(tokens_per_expert, dtype=np.int64)
    ne = tpe.shape[0]
    T, din = x.shape
    dout = w.shape[0] // ne
    wr = w.reshape(ne, dout, din)

    def qd(v, axis, fmax):
        amax = np.max(np.abs(v), axis=axis, keepdims=True)
        scale = np.maximum(amax, EPS) / fmax
        q = np.clip(v / scale, -fmax, fmax).astype(ml_dtypes.float8_e4m3fn)
        return q.astype(np.float32) * scale

    w_dq = qd(wr, (1, 2), E4M3_MAX)
    x_dq = qd(x, -1, E4M3_MAX)
    offs = np.cumsum(tpe)
    starts = offs - tpe
    out = np.zeros((T, dout), np.float32)
    for e in range(ne):
        s, t = int(starts[e]), int(offs[e])
        if t > s:
            out[s:t] = x_dq[s:t] @ w_dq[e].T
    return out.astype(ml_dtypes.bfloat16)


def kernel(x, weight, tokens_per_expert):
    x = np.asarray(x)
    weight = np.asarray(weight)
    tpe = np.asarray(tokens_per_expert)
    uniform = (
        x.shape == (NE * TPE, DIN)
        and weight.shape == (NE * DOUT, DIN)
        and tpe.shape == (NE,)
        and bool(np.all(tpe.astype(np.int64) == TPE))
    )
    if not uniform:
        return _host_reference(x, weight, tpe)

    from concourse.bass_utils import run_bass_kernel_spmd

    nc = get_nc()
    in_maps = make_in_maps(x, weight)
    try:
        res = run_bass_kernel_spmd(nc, in_maps, core_ids=list(range(NE)))
    except Exception:
        # rare device wedge (NRT_EXEC_UNIT_UNRECOVERABLE) — reset and retry
        _axon_device_reset()
        res = run_bass_kernel_spmd(nc, in_maps, core_ids=list(range(NE)))
    return np.concatenate([res.results[e]["o"] for e in range(NE)], axis=0)


if __name__ == "__main__":
    rng = np.random.default_rng(0)
    x = rng.standard_normal((NE * TPE, DIN), dtype=np.float32)
    w = (rng.standard_normal((NE * DOUT, DIN), dtype=np.float32) * 0.02).astype(
        np.float32
    )
    tpe = np.full((NE,), TPE, dtype=np.int32)
    out = kernel(x, w, tpe)
    exp = _host_reference(x, w, tpe)
    a = out.astype(np.float64)
    b = exp.astype(np.float64)
    denom = max(np.abs(b).max(), 1e-30)
    print("absmax rel err:", np.abs(a - b).max() / denom)
    rms = np.sqrt(((a - b) ** 2).mean()) / np.sqrt((b**2).mean())
    print("rms rel err:", rms)



# revision 21
# speedup vs baseline: 1.1888x; 1.1888x over previous
"""ChannelWiseFloat8GroupedLinear — expert-parallel Trainium2 Bass kernel.

Problem: x [8192, 1024] f32, weight [8*1024, 1024] f32, tokens_per_expert
[8] int32 (uniform 1024).  out[t, d] = x_dq @ w_dq[e(t)].T in bf16, where
x is fp8-e4m3fn quant-dequantized per token row and w per expert block.

Sharding: expert-parallel over 8 NeuronCores.  Tokens are contiguous per
expert (cumsum offsets), so core e owns x rows [1024e, 1024e+1024) and
expert e's weight block — no cross-core communication.

Device math: the reference quantizes to OCP e4m3fn (max 448); TRN2's
fp8_e4m3 tops out at 240.  Quantizing with r = 224/amax instead of
448/amax lands on the halved e4m3fn grid, which TRN e4m3 represents
exactly (up to a negligible subnormal-spacing difference), and the x4
is folded into the output scale m[t] = amax_x[t]*amax_w/(448*448/4).
The fp8 matmul accumulates exact products in f32 PSUM, so the result
matches the reference to ~f32 rounding before the final bf16 cast.
"""

import numpy as np
import ml_dtypes

P = 128
TPE = 1024   # tokens per expert (= T // ne, uniform)
DIN = 1024
DOUT = 1024
NE = 8
NT = TPE // P    # 8 token tiles per core
ND = DOUT // P   # 8 dout tiles per core
NK = DIN // P    # 8 contraction tiles
E4M3_MAX = 448.0
EPS = 1e-12

_CACHE = {}


def _axon_device_reset():
    """Best-effort reset of the axon-tunneled NeuronCores after an
    NRT_EXEC_UNIT_UNRECOVERABLE wedge (observed rarely; a reset recovers)."""
    try:
        import ctypes

        import jax

        jax.devices()
        lib = ctypes.CDLL("/opt/axon/libaxon_pjrt.so")
        if hasattr(lib, "axon_reset"):
            lib.axon_reset.restype = ctypes.c_int64
            lib.axon_reset()
    except Exception:
        pass


def _build_nc():
    """Build + compile the single-core Bass program (run SPMD on 8 cores)."""
    import concourse.mybir as mybir
    import concourse.tile as tile
    from concourse import bacc, bass_isa
    from concourse.masks import make_identity

    dt = mybir.dt
    X = mybir.AxisListType.X
    ALU = mybir.AluOpType

    nc = bacc.Bacc("TRN2", target_bir_lowering=False, debug=False)
    x_t = nc.dram_tensor("x", [TPE, DIN], dt.float32, kind="ExternalInput")
    w_t = nc.dram_tensor("w", [DOUT, DIN], dt.float32, kind="ExternalInput")
    o_t = nc.dram_tensor("o", [TPE, DOUT], dt.bfloat16, kind="ExternalOutput")

    x_d = x_t.ap().rearrange("(tt p) k -> p tt k", p=P)   # [128, 8, 1024]
    w_d = w_t.ap().rearrange("(dd p) k -> p dd k", p=P)
    o_d = o_t.ap().rearrange("(tt p) d -> p tt d", p=P)

    with tile.TileContext(nc) as tc:
        with (
            tc.tile_pool(name="const", bufs=1) as const,
            tc.tile_pool(name="big", bufs=1) as big,
            tc.tile_pool(name="small", bufs=1) as small,
            tc.tile_pool(name="outp", bufs=3) as outp,
            tc.tile_pool(name="pt", bufs=2, space="PSUM") as pt,
            tc.tile_pool(name="pm", bufs=2, space="PSUM") as pm,
        ):
            # persistent buffers
            x_sb = big.tile([P, NT, DIN], dt.float32, tag="x_sb")
            w_sb = big.tile([P, ND, DIN], dt.float32, tag="w_sb")
            qx = big.tile([P, NT, DIN], dt.float8e4, tag="qx")
            wT = big.tile([P, NK, ND, P], dt.float32, tag="wT")
            qxT = big.tile([P, NK, NT, P], dt.float8e4, tag="qxT")
            qwT = big.tile([P, NK, ND, P], dt.float8e4, tag="qwT")

            amw_parts = small.tile([P, ND], dt.float32, tag="amw_parts")
            amw_c = small.tile([P, 1], dt.float32, tag="amw_c")
            amw_g = small.tile([P, 1], dt.float32, tag="amw_g")
            inv_w = small.tile([P, 1], dt.float32, tag="inv_w")
            rw = small.tile([P, 1], dt.float32, tag="rw")
            cw = small.tile([P, 1], dt.float32, tag="cw")
            amx_parts = small.tile([P, NT], dt.float32, tag="amx_parts")
            amx_cl = small.tile([P, NT], dt.float32, tag="amx_cl")
            inv_x = small.tile([P, NT], dt.float32, tag="inv_x")
            rx = small.tile([P, NT], dt.float32, tag="rx")
            m_all = small.tile([P, NT], dt.float32, tag="m_all")

            # --- loads first: w first (its global amax gates the w pipeline).
            # 0.5MB chunks: smaller completion-receipt lag on each tile. ---
            for i in range(ND):
                nc.sync.dma_start(w_sb[:, i, :], w_d[:, i, :])
            for i in range(NT):
                nc.sync.dma_start(x_sb[:, i, :], x_d[:, i, :])

            # transpose identity (fp8: 1.0 is exactly representable)
            id_f32 = const.tile([P, P], dt.float32, tag="id32f")
            make_identity(nc, id_f32[:])
            id_fp8 = const.tile([P, P], dt.float8e4, tag="id8")
            nc.vector.tensor_copy(id_fp8[:], id_f32[:])

            # --- amax reduces first: they get engine priority so the
            # per-tile reductions run the moment each DMA lands ---
            for dd in range(ND):
                nc.vector.reduce_max(
                    amw_parts[:, dd : dd + 1],
                    w_sb[:, dd, :],
                    axis=X,
                    apply_absolute_value=True,
                )
            nc.vector.reduce_max(amw_c[:], amw_parts[:], axis=X)
            nc.vector.tensor_scalar_max(amw_c[:], amw_c[:], EPS)
            nc.gpsimd.partition_all_reduce(
                amw_g[:], amw_c[:], channels=P, reduce_op=bass_isa.ReduceOp.max
            )
            nc.vector.reciprocal(inv_w[:], amw_g[:])
            nc.vector.tensor_scalar_mul(rw[:], inv_w[:], E4M3_MAX / 2.0)
            nc.vector.tensor_scalar_mul(cw[:], amw_g[:], 4.0 / (E4M3_MAX * E4M3_MAX))


            # --- w: exact f32 transpose (PE transpose-mode) during the load
            # window (no amax dependency), staged to wT in f32 ---
            for dd in range(ND):
                pwf = pt.tile([P, NK, P], dt.float32, tag="pt")
                for kk in range(NK):
                    nc.tensor.transpose(
                        pwf[:, kk, :], w_sb[:, dd, kk * P : (kk + 1) * P], id_f32[:]
                    )
                nc.scalar.copy(wT[:, :, dd, :], pwf[:])

            # --- w quantize per (kk, dd-half), progressive: first halves
            # unblock after the dd0-3 copies, and the main matmul consumes
            # qwT in kk order ---
            for kk in range(NK):
                for h in range(2):
                    hsl = slice(h * ND // 2, (h + 1) * ND // 2)
                    nc.scalar.mul(qwT[:, kk, hsl, :], wT[:, kk, hsl, :], rw[:])

            # --- x: per-pair chain (amax -> scales -> quant -> PE transpose
            # -> evict), emission-ordered so engine priorities follow the
            # dependency chain ---
            for pair in range(4):
                t0 = 2 * pair
                sl = slice(t0, t0 + 2)
                for tt in (t0, t0 + 1):
                    nc.vector.reduce_max(
                        amx_parts[:, tt : tt + 1],
                        x_sb[:, tt, :],
                        axis=X,
                        apply_absolute_value=True,
                    )
                nc.vector.tensor_scalar_max(amx_cl[:, sl], amx_parts[:, sl], EPS)
                nc.vector.reciprocal(inv_x[:, sl], amx_cl[:, sl])
                nc.vector.tensor_scalar_mul(rx[:, sl], inv_x[:, sl], E4M3_MAX / 2.0)
                nc.vector.tensor_scalar(
                    m_all[:, sl], amx_cl[:, sl], cw[:], None, op0=ALU.mult
                )
                for tt in (t0, t0 + 1):
                    nc.vector.tensor_scalar_mul(
                        qx[:, tt, :], x_sb[:, tt, :], rx[:, tt : tt + 1]
                    )
                    pxf = pt.tile([P, NK, P], dt.float32, tag="pt")
                    for kk in range(NK):
                        nc.tensor.matmul(
                            pxf[:, kk, :],
                            lhsT=qx[:, tt, kk * P : (kk + 1) * P],
                            rhs=id_fp8[:],
                            start=True, stop=True,
                        )
                    if tt % 2 == 0:
                        nc.vector.tensor_copy(qxT[:, :, tt, :], pxf[:])
                    else:
                        nc.scalar.copy(qxT[:, :, tt, :], pxf[:])

            # --- main fp8 matmul: out[t,d] accumulated over k, DoubleRow
            # (2 fp8 weights per PE cell -> two k-tiles per pass, ~1.4x) ---
            DR = mybir.MatmulPerfMode.DoubleRow
            for tt in range(NT):
                po = pm.tile([P, DOUT], dt.float32, tag="pm")
                for kp in range(NK // 2):
                    ks = slice(2 * kp, 2 * kp + 2)
                    st, sp = kp == 0, kp == NK // 2 - 1
                    nc.tensor.matmul(
                        po[:, 0 : DOUT // 2],
                        lhsT=qxT[:, ks, tt, :],
                        rhs=qwT[:, ks, 0 : ND // 2, :],
                        start=st, stop=sp, perf_mode=DR,
                    )
                    nc.tensor.matmul(
                        po[:, DOUT // 2 : DOUT],
                        lhsT=qxT[:, ks, tt, :],
                        rhs=qwT[:, ks, ND // 2 : ND, :],
                        start=st, stop=sp, perf_mode=DR,
                    )
                ob = outp.tile([P, DOUT], dt.bfloat16, tag="ob")
                if tt % 2 == 0:
                    nc.vector.tensor_scalar_mul(ob[:], po[:], m_all[:, tt : tt + 1])
                else:
                    nc.scalar.mul(ob[:], po[:], m_all[:, tt : tt + 1])
                nc.sync.dma_start(o_d[:, tt, :], ob[:])

    nc.compile()
    return nc


def get_nc():
    if "nc" not in _CACHE:
        _CACHE["nc"] = _build_nc()
    return _CACHE["nc"]


def make_in_maps(x, weight):
    x = np.ascontiguousarray(np.asarray(x, dtype=np.float32))
    w = np.ascontiguousarray(np.asarray(weight, dtype=np.float32))
    return [
        {"x": x[TPE * e : TPE * (e + 1)], "w": w[DOUT * e : DOUT * (e + 1)]}
        for e in range(NE)
    ]


def _host_reference(x, weight, tokens_per_expert):
    """Exact numpy port of the reference — fallback for non-uniform routing."""
    x = np.asarray(x, dtype=np.float32)
    w = np.asarray(weight, dtype=np.float32)
    tpe = np.asarray(tokens_per_expert, dtype=np.int64)
    ne = tpe.shape[0]
    T, din = x.shape
    dout = w.shape[0] // ne
    wr = w.reshape(ne, dout, din)

    def qd(v, axis, fmax):
        amax = np.max(np.abs(v), axis=axis, keepdims=True)
        scale = np.maximum(amax, EPS) / fmax
        q = np.clip(v / scale, -fmax, fmax).astype(ml_dtypes.float8_e4m3fn)
        return q.astype(np.float32) * scale

    w_dq = qd(wr, (1, 2), E4M3_MAX)
    x_dq = qd(x, -1, E4M3_MAX)
    offs = np.cumsum(tpe)
    starts = offs - tpe
    out = np.zeros((T, dout), np.float32)
    for e in range(ne):
        s, t = int(starts[e]), int(offs[e])
        if t > s:
            out[s:t] = x_dq[s:t] @ w_dq[e].T
    return out.astype(ml_dtypes.bfloat16)


def kernel(x, weight, tokens_per_expert):
    x = np.asarray(x)
    weight = np.asarray(weight)
    tpe = np.asarray(tokens_per_expert)
    uniform = (
        x.shape == (NE * TPE, DIN)
        and weight.shape == (NE * DOUT, DIN)
        and tpe.shape == (NE,)
        and bool(np.all(tpe.astype(np.int64) == TPE))
    )
    if not uniform:
        return _host_reference(x, weight, tpe)

    from concourse.bass_utils import run_bass_kernel_spmd

    nc = get_nc()
    in_maps = make_in_maps(x, weight)
    try:
        res = run_bass_kernel_spmd(nc, in_maps, core_ids=list(range(NE)))
    except Exception:
        # rare device wedge (NRT_EXEC_UNIT_UNRECOVERABLE) — reset and retry
        _axon_device_reset()
        res = run_bass_kernel_spmd(nc, in_maps, core_ids=list(range(NE)))
    return np.concatenate([res.results[e]["o"] for e in range(NE)], axis=0)


if __name__ == "__main__":
    rng = np.random.default_rng(0)
    x = rng.standard_normal((NE * TPE, DIN), dtype=np.float32)
    w = (rng.standard_normal((NE * DOUT, DIN), dtype=np.float32) * 0.02).astype(
        np.float32
    )
    tpe = np.full((NE,), TPE, dtype=np.int32)
    out = kernel(x, w, tpe)
    exp = _host_reference(x, w, tpe)
    a = out.astype(np.float64)
    b = exp.astype(np.float64)
    denom = max(np.abs(b).max(), 1e-30)
    print("absmax rel err:", np.abs(a - b).max() / denom)
    rms = np.sqrt(((a - b) ** 2).mean()) / np.sqrt((b**2).mean())
    print("rms rel err:", rms)



# revision 22
# speedup vs baseline: 1.4010x; 1.1785x over previous
"""ChannelWiseFloat8GroupedLinear — expert-parallel Trainium2 Bass kernel.

Problem: x [8192, 1024] f32, weight [8*1024, 1024] f32, tokens_per_expert
[8] int32 (uniform 1024).  out[t, d] = x_dq @ w_dq[e(t)].T in bf16, where
x is fp8-e4m3fn quant-dequantized per token row and w per expert block.

Sharding: expert-parallel over 8 NeuronCores.  Tokens are contiguous per
expert (cumsum offsets), so core e owns x rows [1024e, 1024e+1024) and
expert e's weight block — no cross-core communication.

Device math: the reference quantizes to OCP e4m3fn (max 448); TRN2's
fp8_e4m3 tops out at 240.  Quantizing with r = 224/amax instead of
448/amax lands on the halved e4m3fn grid, which TRN e4m3 represents
exactly (up to a negligible subnormal-spacing difference), and the x4
is folded into the output scale m[t] = amax_x[t]*amax_w/(448*448/4).
The fp8 matmul accumulates exact products in f32 PSUM, so the result
matches the reference to ~f32 rounding before the final bf16 cast.
"""

import numpy as np
import ml_dtypes

P = 128
TPE = 1024   # tokens per expert (= T // ne, uniform)
DIN = 1024
DOUT = 1024
NE = 8
NT = TPE // P    # 8 token tiles per core
ND = DOUT // P   # 8 dout tiles per core
NK = DIN // P    # 8 contraction tiles
E4M3_MAX = 448.0
EPS = 1e-12

_CACHE = {}


def _axon_device_reset():
    """Best-effort reset of the axon-tunneled NeuronCores after an
    NRT_EXEC_UNIT_UNRECOVERABLE wedge (observed rarely; a reset recovers)."""
    try:
        import ctypes

        import jax

        jax.devices()
        lib = ctypes.CDLL("/opt/axon/libaxon_pjrt.so")
        if hasattr(lib, "axon_reset"):
            lib.axon_reset.restype = ctypes.c_int64
            lib.axon_reset()
    except Exception:
        pass


def _build_nc():
    """Build + compile the single-core Bass program (run SPMD on 8 cores)."""
    import concourse.mybir as mybir
    import concourse.tile as tile
    from concourse import bacc, bass_isa
    from concourse.masks import make_identity

    dt = mybir.dt
    X = mybir.AxisListType.X
    ALU = mybir.AluOpType

    nc = bacc.Bacc("TRN2", target_bir_lowering=False, debug=False)
    x_t = nc.dram_tensor("x", [TPE, DIN], dt.float32, kind="ExternalInput")
    w_t = nc.dram_tensor("w", [DOUT, DIN], dt.float32, kind="ExternalInput")
    o_t = nc.dram_tensor("o", [TPE, DOUT], dt.bfloat16, kind="ExternalOutput")

    x_d = x_t.ap().rearrange("(tt p) k -> p tt k", p=P)   # [128, 8, 1024]
    w_d = w_t.ap().rearrange("(dd p) k -> p dd k", p=P)
    o_d = o_t.ap().rearrange("(tt p) d -> p tt d", p=P)

    with tile.TileContext(nc) as tc:
        with (
            tc.tile_pool(name="const", bufs=1) as const,
            tc.tile_pool(name="big", bufs=1) as big,
            tc.tile_pool(name="small", bufs=1) as small,
            tc.tile_pool(name="outp", bufs=3) as outp,
            tc.tile_pool(name="pt", bufs=2, space="PSUM") as pt,
            tc.tile_pool(name="pm", bufs=2, space="PSUM") as pm,
        ):
            # persistent buffers
            x_sb = big.tile([P, NT, DIN], dt.float32, tag="x_sb")
            w_sb = big.tile([P, ND, DIN], dt.float32, tag="w_sb")
            qx = big.tile([P, NT, DIN], dt.float8e4, tag="qx")
            wT = big.tile([P, NK, ND, P], dt.float32, tag="wT")
            qxT = big.tile([P, NK, NT, P], dt.float8e4, tag="qxT")
            qwT = big.tile([P, NK, ND, P], dt.float8e4, tag="qwT")

            amw_parts = small.tile([P, ND], dt.float32, tag="amw_parts")
            amw_c = small.tile([P, 1], dt.float32, tag="amw_c")
            amw_g = small.tile([P, 1], dt.float32, tag="amw_g")
            inv_w = small.tile([P, 1], dt.float32, tag="inv_w")
            rw = small.tile([P, 1], dt.float32, tag="rw")
            cw = small.tile([P, 1], dt.float32, tag="cw")
            amx_parts = small.tile([P, NT], dt.float32, tag="amx_parts")
            amx_cl = small.tile([P, NT], dt.float32, tag="amx_cl")
            inv_x = small.tile([P, NT], dt.float32, tag="inv_x")
            rx = small.tile([P, NT], dt.float32, tag="rx")
            m_all = small.tile([P, NT], dt.float32, tag="m_all")

            # --- loads first: w first (its global amax gates the w pipeline).
            # 0.5MB chunks: smaller completion-receipt lag on each tile. ---
            for i in range(ND):
                nc.sync.dma_start(w_sb[:, i, :], w_d[:, i, :])
            for i in range(NT):
                nc.sync.dma_start(x_sb[:, i, :], x_d[:, i, :])

            # transpose identity (fp8: 1.0 is exactly representable)
            id_f32 = const.tile([P, P], dt.float32, tag="id32f")
            make_identity(nc, id_f32[:])
            id_fp8 = const.tile([P, P], dt.float8e4, tag="id8")
            nc.vector.tensor_copy(id_fp8[:], id_f32[:])

            # --- amax reduces first: they get engine priority so the
            # per-tile reductions run the moment each DMA lands ---
            for dd in range(ND):
                nc.vector.reduce_max(
                    amw_parts[:, dd : dd + 1],
                    w_sb[:, dd, :],
                    axis=X,
                    apply_absolute_value=True,
                )
            nc.vector.reduce_max(amw_c[:], amw_parts[:], axis=X)
            nc.vector.tensor_scalar_max(amw_c[:], amw_c[:], EPS)
            nc.gpsimd.partition_all_reduce(
                amw_g[:], amw_c[:], channels=P, reduce_op=bass_isa.ReduceOp.max
            )
            nc.vector.reciprocal(inv_w[:], amw_g[:])
            nc.vector.tensor_scalar_mul(rw[:], inv_w[:], E4M3_MAX / 2.0)
            nc.vector.tensor_scalar_mul(cw[:], amw_g[:], 4.0 / (E4M3_MAX * E4M3_MAX))


            # --- w: exact f32 transpose (PE transpose-mode) during the load
            # window (no amax dependency), staged to wT in f32 ---
            for dd in range(ND):
                pwf = pt.tile([P, NK, P], dt.float32, tag="pt")
                for kk in range(NK):
                    nc.tensor.transpose(
                        pwf[:, kk, :], w_sb[:, dd, kk * P : (kk + 1) * P], id_f32[:]
                    )
                nc.scalar.copy(wT[:, :, dd, :], pwf[:])

            # --- w quantize per kk-PAIR (FD=2048): everything is gated on
            # rw anyway, so fewer/bigger ACT ops shorten the serial quant
            # chain (~450ns fixed cost per op saved), and each op completes
            # exactly one DoubleRow k-pair in consumption order ---
            for kp in range(NK // 2):
                ks = slice(2 * kp, 2 * kp + 2)
                nc.scalar.mul(qwT[:, ks, :, :], wT[:, ks, :, :], rw[:])

            # --- x: per-pair chain (amax -> scales -> quant -> PE transpose
            # -> evict), emission-ordered so engine priorities follow the
            # dependency chain ---
            for pair in range(4):
                t0 = 2 * pair
                sl = slice(t0, t0 + 2)
                for tt in (t0, t0 + 1):
                    nc.vector.reduce_max(
                        amx_parts[:, tt : tt + 1],
                        x_sb[:, tt, :],
                        axis=X,
                        apply_absolute_value=True,
                    )
                nc.vector.tensor_scalar_max(amx_cl[:, sl], amx_parts[:, sl], EPS)
                nc.vector.reciprocal(inv_x[:, sl], amx_cl[:, sl])
                nc.vector.tensor_scalar_mul(rx[:, sl], inv_x[:, sl], E4M3_MAX / 2.0)
                nc.vector.tensor_scalar(
                    m_all[:, sl], amx_cl[:, sl], cw[:], None, op0=ALU.mult
                )
                for tt in (t0, t0 + 1):
                    nc.vector.tensor_scalar_mul(
                        qx[:, tt, :], x_sb[:, tt, :], rx[:, tt : tt + 1]
                    )
                    pxf = pt.tile([P, NK, P], dt.float32, tag="pt")
                    for kk in range(NK):
                        nc.tensor.matmul(
                            pxf[:, kk, :],
                            lhsT=qx[:, tt, kk * P : (kk + 1) * P],
                            rhs=id_fp8[:],
                            start=True, stop=True,
                        )
                    if tt % 2 == 0:
                        nc.vector.tensor_copy(qxT[:, :, tt, :], pxf[:])
                    else:
                        nc.scalar.copy(qxT[:, :, tt, :], pxf[:])

            # --- main fp8 matmul: out[t,d] accumulated over k, DoubleRow
            # (2 fp8 weights per PE cell -> two k-tiles per pass, ~1.4x) ---
            DR = mybir.MatmulPerfMode.DoubleRow
            for tt in range(NT):
                po = pm.tile([P, DOUT], dt.float32, tag="pm")
                for kp in range(NK // 2):
                    ks = slice(2 * kp, 2 * kp + 2)
                    st, sp = kp == 0, kp == NK // 2 - 1
                    nc.tensor.matmul(
                        po[:, 0 : DOUT // 2],
                        lhsT=qxT[:, ks, tt, :],
                        rhs=qwT[:, ks, 0 : ND // 2, :],
                        start=st, stop=sp, perf_mode=DR,
                    )
                    nc.tensor.matmul(
                        po[:, DOUT // 2 : DOUT],
                        lhsT=qxT[:, ks, tt, :],
                        rhs=qwT[:, ks, ND // 2 : ND, :],
                        start=st, stop=sp, perf_mode=DR,
                    )
                ob = outp.tile([P, DOUT], dt.bfloat16, tag="ob")
                if tt % 2 == 0:
                    nc.vector.tensor_scalar_mul(ob[:], po[:], m_all[:, tt : tt + 1])
                else:
                    nc.scalar.mul(ob[:], po[:], m_all[:, tt : tt + 1])
                nc.sync.dma_start(o_d[:, tt, :], ob[:])

    nc.compile()
    return nc


def get_nc():
    if "nc" not in _CACHE:
        _CACHE["nc"] = _build_nc()
    return _CACHE["nc"]


def make_in_maps(x, weight):
    x = np.ascontiguousarray(np.asarray(x, dtype=np.float32))
    w = np.ascontiguousarray(np.asarray(weight, dtype=np.float32))
    return [
        {"x": x[TPE * e : TPE * (e + 1)], "w": w[DOUT * e : DOUT * (e + 1)]}
        for e in range(NE)
    ]


def _host_reference(x, weight, tokens_per_expert):
    """Exact numpy port of the reference — fallback for non-uniform routing."""
    x = np.asarray(x, dtype=np.float32)
    w = np.asarray(weight, dtype=np.float32)
    tpe = np.asarray(tokens_per_expert, dtype=np.int64)
    ne = tpe.shape[0]
    T, din = x.shape
    dout = w.shape[0] // ne
    wr = w.reshape(ne, dout, din)

    def qd(v, axis, fmax):
        amax = np.max(np.abs(v), axis=axis, keepdims=True)
        scale = np.maximum(amax, EPS) / fmax
        q = np.clip(v / scale, -fmax, fmax).astype(ml_dtypes.float8_e4m3fn)
        return q.astype(np.float32) * scale

    w_dq = qd(wr, (1, 2), E4M3_MAX)
    x_dq = qd(x, -1, E4M3_MAX)
    offs = np.cumsum(tpe)
    starts = offs - tpe
    out = np.zeros((T, dout), np.float32)
    for e in range(ne):
        s, t = int(starts[e]), int(offs[e])
        if t > s:
            out[s:t] = x_dq[s:t] @ w_dq[e].T
    return out.astype(ml_dtypes.bfloat16)


def kernel(x, weight, tokens_per_expert):
    x = np.asarray(x)
    weight = np.asarray(weight)
    tpe = np.asarray(tokens_per_expert)
    uniform = (
        x.shape == (NE * TPE, DIN)
        and weight.shape == (NE * DOUT, DIN)
        and tpe.shape == (NE,)
        and bool(np.all(tpe.astype(np.int64) == TPE))
    )
    if not uniform:
        return _host_reference(x, weight, tpe)

    from concourse.bass_utils import run_bass_kernel_spmd

    nc = get_nc()
    in_maps = make_in_maps(x, weight)
    try:
        res = run_bass_kernel_spmd(nc, in_maps, core_ids=list(range(NE)))
    except Exception:
        # rare device wedge (NRT_EXEC_UNIT_UNRECOVERABLE) — reset and retry
        _axon_device_reset()
        res = run_bass_kernel_spmd(nc, in_maps, core_ids=list(range(NE)))
    return np.concatenate([res.results[e]["o"] for e in range(NE)], axis=0)


if __name__ == "__main__":
    rng = np.random.default_rng(0)
    x = rng.standard_normal((NE * TPE, DIN), dtype=np.float32)
    w = (rng.standard_normal((NE * DOUT, DIN), dtype=np.float32) * 0.02).astype(
        np.float32
    )
    tpe = np.full((NE,), TPE, dtype=np.int32)
    out = kernel(x, w, tpe)
    exp = _host_reference(x, w, tpe)
    a = out.astype(np.float64)
    b = exp.astype(np.float64)
    denom = max(np.abs(b).max(), 1e-30)
    print("absmax rel err:", np.abs(a - b).max() / denom)
    rms = np.sqrt(((a - b) ** 2).mean()) / np.sqrt((b**2).mean())
    print("rms rel err:", rms)

